# revision 8
# baseline (speedup 1.0000x reference)
"""AdaptiveGridMerger Trainium2 kernel.

Math: the reference scatters x[b,c,:] into a flat 8x8 grid with bilinear
(4-corner) weights from positions[b,c,:], then matmuls grid_weights
GW [270,64]. The scatter matrix S_b [64,306] (column c = the bilinear
hat weights of channel c) is tiny and depends only on positions, so it
is built on the HOST. The tail output rows 256:270 are folded into it:
  st78[c, 0:64]  = S_b[:, c]
  st78[c, 64:78] = (S_b.T @ GW[256:270].T)[c]   (Wtail fold)
so mm1 (lhsT=st78) produces gv[0:64] = S@x AND gv[64:78] = out[256:270]
in one pass. mm2 (lhsT=GW[0:256].T) produces out[0:256] from gv[0:64].

Device work: 6 contiguous [128,*] read DMAs on the sync HWDGE ring
(strict FIFO = reads drain at full HBM rate before any write), bf16
matmuls, PSUM->SBUF cast copies alternating DVE/ACT, write DMAs on the
sync ring in readiness order. st and gw ride as extra columns of the
xc1 read (a separate small read is latency-bound and stalls everything).

The contraction accumulates in arrival order c1 -> c0 -> c2; work is
pipelined in (batch, T-half) units so mm1 of one unit holds only 4
PSUM banks while the previous unit's mm2 rotates through the other 4 —
the PE never waits long on the 2-wide PSUM-evacuation copy stream.
Sharding: data-parallel over batch, 2 batches per core. Spin matmuls
pre-ramp the PE clock during the DMA lead-in.
"""

import numpy as np

import concourse.bass as bass
import concourse.bacc as bacc
import concourse.mybir as mybir
from concourse import tile
from concourse.bass_utils import run_bass_kernel_spmd

B, C, T = 16, 306, 4096
M, G, GS = 270, 64, 8
N_CORES = 8
BL = B // N_CORES  # batches per core

W78 = G + 14          # st block width: 64 grid cols + 14 folded tail cols
XC = T // 2
SB = 3 * W78          # st cols per batch (c0, c1, c2-packed)
SE0 = T               # st base col inside xc1e
GE0 = T + SB          # gw base col inside xc1e
WX1 = GE0 + 256       # xc1e width
T_PS = 512
N_SPIN = 8

MM_DTYPE = mybir.dt.bfloat16
NP_MM = mybir.dt.np(MM_DTYPE)
FP32 = mybir.dt.float32


def build_nc():
    nc = bacc.Bacc()
    x2_ext = nc.declare_dram_parameter("x2", [BL, 128, XC], MM_DTYPE, isOutput=False)
    xc1e_ext = nc.declare_dram_parameter("xc1e", [BL, 128, WX1], MM_DTYPE, isOutput=False)
    xc0_ext = nc.declare_dram_parameter("xc0", [BL, 128, T], MM_DTYPE, isOutput=False)
    out_ext = nc.declare_dram_parameter("out", [BL, M, T], MM_DTYPE, isOutput=True)

    with tile.TileContext(nc) as tc:
        with (
            tc.tile_pool(name="const", bufs=1) as constp,
            tc.tile_pool(name="xp", bufs=1) as xp,
            tc.tile_pool(name="gvt", bufs=2) as gvtp,
            tc.tile_pool(name="op", bufs=2) as outp,
            tc.tile_pool(name="ps", bufs=4, space=bass.MemorySpace.PSUM) as psp,
        ):
            # PE clock pre-ramp while the first reads stream in.
            dummy = constp.tile([128, T_PS], MM_DTYPE, tag="dummy")
            nc.vector.memset(dummy[:], 0.0)
            spin_ps = psp.tile([128, 2 * T_PS], FP32, tag="pb", name="spin_ps")
            for _ in range(N_SPIN):
                nc.tensor.matmul(
                    spin_ps[:, :T_PS], dummy[:, :128], dummy[:], start=True, stop=True
                )

            # ---- x reads on the sync ring, in consumption order
            x2 = {}
            xc0 = {}
            xc1 = {}
            for b in range(BL):
                xc1[b] = xp.tile([128, WX1], MM_DTYPE, tag=f"xc1_{b}", name=f"xc1_{b}")
                nc.sync.dma_start(out=xc1[b][:], in_=xc1e_ext[b])
                xc0[b] = xp.tile([128, T], MM_DTYPE, tag=f"xc0_{b}", name=f"xc0_{b}")
                nc.sync.dma_start(out=xc0[b][:], in_=xc0_ext[b])
                x2[b] = xp.tile([128, XC], MM_DTYPE, tag=f"x2_{b}", name=f"x2_{b}")
                nc.sync.dma_start(out=x2[b][:], in_=x2_ext[b])

            k_state = {"k": 0}

            def evac(dst, src):
                if k_state["k"] % 2 == 0:
                    nc.vector.tensor_copy(dst, src)
                else:
                    nc.scalar.copy(dst, src)
                k_state["k"] += 1

            out_sb = {}
            for b in range(BL):
                for mi in range(2):
                    out_sb[(b, mi)] = outp.tile(
                        [128, T], MM_DTYPE, tag=f"o{mi}", name=f"o{b}_{mi}"
                    )
            gvts = {}
            for b in range(BL):
                gvts[b] = gvtp.tile([W78, T], MM_DTYPE, tag="gvt", name=f"gvt{b}")

            def mm1_unit(b, tt):
                """One (batch, T-half) unit: c1 -> c0 -> c2 into 2 gv tiles,
                then evacuate both to gvt."""
                gv = [
                    psp.tile([128, 2 * T_PS], FP32, tag="pb", name=f"gv{b}_{tt}{h}")
                    for h in range(2)
                ]
                base = tt * 4 * T_PS
                for which, start, stop in ((1, True, False), (0, False, False)):
                    src = xc1[b] if which == 1 else xc0[b]
                    lhs = xc1[b][0:128, SE0 + which * W78 : SE0 + (which + 1) * W78]
                    for q in range(4):
                        f0 = base + q * T_PS
                        nc.tensor.matmul(
                            gv[q // 2][:W78, (q % 2) * T_PS : (q % 2 + 1) * T_PS],
                            lhs,
                            src[:, f0 : f0 + T_PS],
                            start=start,
                            stop=stop,
                            skip_group_check=True,
                        )
                # c2: packed rows 0:50 (tt0) / 64:114 (tt1) of the single block
                p0 = 0 if tt == 0 else 64
                lhs = xc1[b][p0 : p0 + 50, SE0 + 2 * W78 : SE0 + 3 * W78]
                for q in range(4):
                    nc.tensor.matmul(
                        gv[q // 2][:W78, (q % 2) * T_PS : (q % 2 + 1) * T_PS],
                        lhs,
                        x2[b][p0 : p0 + 50, q * T_PS : (q + 1) * T_PS],
                        start=False,
                        stop=True,
                        skip_group_check=True,
                    )
                for h in range(2):
                    evac(
                        gvts[b][:W78, base + h * 2 * T_PS : base + (h + 1) * 2 * T_PS],
                        gv[h][:W78],
                    )

            def mm2_unit(b, tt):
                for mi in range(2):
                    for h in range(2):
                        o_ps = psp.tile([128, 2 * T_PS], FP32, tag="pb", name="o_ps")
                        c0 = tt * 4 * T_PS + h * 2 * T_PS
                        for q in range(2):
                            nc.tensor.matmul(
                                o_ps[:, q * T_PS : (q + 1) * T_PS],
                                xc1[b][0:G, GE0 + mi * 128 : GE0 + (mi + 1) * 128],
                                gvts[b][:G, c0 + q * T_PS : c0 + (q + 1) * T_PS],
                                start=True,
                                stop=True,
                                skip_group_check=True,
                            )
                        evac(out_sb[(b, mi)][:, c0 : c0 + 2 * T_PS], o_ps[:])
                for mi in range(2):
                    nc.sync.dma_start(
                        out=out_ext[b, mi * 128 : (mi + 1) * 128, tt * XC : (tt + 1) * XC],
                        in_=out_sb[(b, mi)][:, tt * XC : (tt + 1) * XC],
                    )

            # software-pipelined unit schedule: mm1 of unit k+1 overlaps
            # mm2 of unit k (each phase holds 4 of the 8 PSUM banks)
            mm1_unit(0, 0)
            mm1_unit(0, 1)
            nc.sync.dma_start(out=out_ext[0, 256:M, :], in_=gvts[0][G:W78, :])
            mm2_unit(0, 0)
            mm1_unit(1, 0)
            mm2_unit(0, 1)
            mm1_unit(1, 1)
            mm2_unit(1, 0)
            mm2_unit(1, 1)
            nc.sync.dma_start(out=out_ext[1, 256:M, :], in_=gvts[1][G:W78, :])
    nc.compile()
    return nc


def _host_st(positions, grid_weights):
    """st78 [B, C, 78] f32: bilinear hat weights + folded tail rows."""
    gp = (positions.astype(np.float32) + 1.0) * (GS / 2.0)  # [B, C, 2]
    i = np.arange(GS, dtype=np.float32)
    wy = np.maximum(0.0, 1.0 - np.abs(i[None, None, :] - gp[:, :, 0:1]))
    wx = np.maximum(0.0, 1.0 - np.abs(i[None, None, :] - gp[:, :, 1:2]))
    s = (wy[:, :, :, None] * wx[:, :, None, :]).reshape(B, C, G)
    wtail = s @ grid_weights[256:M].T.astype(np.float32)  # [B, C, 14]
    return np.concatenate([s, wtail], axis=2)


def make_in_maps(x, positions, grid_weights):
    st78 = _host_st(positions, grid_weights)
    gw = np.ascontiguousarray(grid_weights[:256].T).astype(NP_MM)  # [64, 256]
    x_mm = x.astype(NP_MM)
    in_maps = []
    for i in range(N_CORES):
        sl = slice(i * BL, (i + 1) * BL)
        xc1e_pack = np.zeros((BL, 128, WX1), dtype=np.float32)
        x2_pack = np.zeros((BL, 128, XC), dtype=NP_MM)
        for b2 in range(BL):
            gb = i * BL + b2
            xc1e_pack[b2, :, 0:T] = x_mm[gb, 128:256].astype(np.float32)
            xc1e_pack[b2, :, SE0 + W78 : SE0 + 2 * W78] = st78[gb, 128:256]
            xc1e_pack[b2, :, SE0 : SE0 + W78] = st78[gb, 0:128]
            # c2 block: tt0 weights at rows 0:50, tt1 weights at rows 64:114
            xc1e_pack[b2, 0:50, SE0 + 2 * W78 : SE0 + 3 * W78] = st78[gb, 256:C]
            xc1e_pack[b2, 64:114, SE0 + 2 * W78 : SE0 + 3 * W78] = st78[gb, 256:C]
            xc1e_pack[b2, 0:64, GE0 : GE0 + 256] = gw
            xc2 = x_mm[gb, 256:C].reshape(50, 2, XC)
            x2_pack[b2, 0:50] = xc2[:, 0]
            x2_pack[b2, 64:114] = xc2[:, 1]
        in_maps.append(
            {
                "xc1e": xc1e_pack.astype(NP_MM),
                "x2": x2_pack,
                "xc0": np.ascontiguousarray(x_mm[sl, 0:128]),
            }
        )
    return in_maps


_NC_CACHE = None


def kernel(x, positions, grid_weights):
    global _NC_CACHE
    if _NC_CACHE is None:
        _NC_CACHE = build_nc()
    nc = _NC_CACHE
    in_maps = make_in_maps(x, positions, grid_weights)
    res = run_bass_kernel_spmd(nc, in_maps, core_ids=list(range(N_CORES)))
    out = np.concatenate([r["out"] for r in res.results], axis=0)
    return np.asarray(out, dtype=np.float32)


if __name__ == "__main__":
    xs = np.random.randn(B, C, T).astype(np.float32)
    ps = np.random.uniform(-1, 0.74, (B, C, 2)).astype(np.float32)
    gw = np.random.randn(M, G).astype(np.float32)
    out = kernel(xs, ps, gw)
    print(out.shape, out.dtype)


# revision 9
# speedup vs baseline: 1.0816x; 1.0816x over previous
"""AdaptiveGridMerger Trainium2 kernel.

Math: the reference scatters x[b,c,:] into a flat 8x8 grid with bilinear
(4-corner) weights from positions[b,c,:], then matmuls grid_weights
GW [270,64]. The scatter matrix S_b [64,306] (column c = the bilinear
hat weights of channel c) is tiny and depends only on positions, so it
is built on the HOST. The tail output rows 256:270 are folded into it:
  st78[c, 0:64]  = S_b[:, c]
  st78[c, 64:78] = (S_b.T @ GW[256:270].T)[c]   (Wtail fold)
so mm1 (lhsT=st78) produces gv[0:64] = S@x AND gv[64:78] = out[256:270]
in one pass. mm2 (lhsT=GW[0:256].T) produces out[0:256] from gv[0:64].

Device work: 10 contiguous [128,*] read DMAs on the sync HWDGE ring
(strict FIFO = reads drain at full HBM rate before any write), bf16
matmuls, PSUM->SBUF cast copies alternating DVE/ACT, write DMAs on the
sync ring in readiness order. st and gw ride as extra columns of the
small x2 read (a separate small read is latency-bound and stalls
everything), so the first matmul dependency is the smallest read.

Per batch the contraction accumulates c2 (start, smallest read, first)
-> c0 -> c1 (stop); the big reads are split into [128,2048] halves so
completion semaphores (which lag data by 1.5-3us) hand off earlier.
Zero-weight accumulate matmuls between phases keep the PE activity
monitor busy while reads stream, so the clock stays at 2.4 GHz.
Sharding: data-parallel over batch, 2 batches per core. PSUM: one pool
of 4 x [128,1024] f32 slots = exactly 8 banks.
"""

import numpy as np

import concourse.bass as bass
import concourse.bacc as bacc
import concourse.mybir as mybir
from concourse import tile
from concourse.bass_utils import run_bass_kernel_spmd

B, C, T = 16, 306, 4096
M, G, GS = 270, 64, 8
N_CORES = 8
BL = B // N_CORES  # batches per core

W78 = G + 14          # st block width: 64 grid cols + 14 folded tail cols
XC = T // 2
SE0 = XC              # st base col inside x2st
GE0 = XC + 3 * W78    # gw base col inside x2st
WX2 = GE0 + 128       # x2st width (2048 + 234 + 128 = 2410)
T_PS = 512
N_SPIN = 7
N_WARM = 8

MM_DTYPE = mybir.dt.bfloat16
NP_MM = mybir.dt.np(MM_DTYPE)
FP32 = mybir.dt.float32


def build_nc():
    nc = bacc.Bacc()
    x2st_ext = nc.declare_dram_parameter("x2st", [BL, 128, WX2], MM_DTYPE, isOutput=False)
    x01_ext = nc.declare_dram_parameter("x01", [BL, 2, 128, T], MM_DTYPE, isOutput=False)
    out_ext = nc.declare_dram_parameter("out", [BL, M, T], MM_DTYPE, isOutput=True)

    with tile.TileContext(nc) as tc:
        with (
            tc.tile_pool(name="const", bufs=1) as constp,
            tc.tile_pool(name="xp", bufs=1) as xp,
            tc.tile_pool(name="gvt", bufs=2) as gvtp,
            tc.tile_pool(name="op", bufs=2) as outp,
            tc.tile_pool(name="ps", bufs=4, space=bass.MemorySpace.PSUM) as psp,
        ):
            # PE clock pre-ramp while the first reads stream in.
            dummy = constp.tile([128, T_PS], MM_DTYPE, tag="dummy")
            nc.vector.memset(dummy[:], 0.0)
            spin_ps = psp.tile([128, 2 * T_PS], FP32, tag="pb", name="spin_ps")
            for _ in range(N_SPIN):
                nc.tensor.matmul(
                    spin_ps[:, :T_PS], dummy[:, :128], dummy[:], start=True, stop=True
                )

            # ---- reads on the sync ring, in consumption order; big chunks
            # ---- split into [128,2048] halves for earlier sem handoff
            x2st = {}
            xh = {}
            for b in range(BL):
                x2st[b] = xp.tile([128, WX2], MM_DTYPE, tag=f"x2st{b}", name=f"x2st{b}")
                nc.sync.dma_start(out=x2st[b][:], in_=x2st_ext[b])
                for ci in range(2):
                    t_ = xp.tile([128, T], MM_DTYPE, tag=f"x{b}{ci}", name=f"x{b}{ci}")
                    for hw in range(2):
                        nc.sync.dma_start(
                            out=t_[:, hw * XC : (hw + 1) * XC],
                            in_=x01_ext[b, ci, :, hw * XC : (hw + 1) * XC],
                        )
                    xh[(b, ci)] = t_

            k_state = {"k": 0}

            def evac(dst, src):
                if k_state["k"] % 2 == 0:
                    nc.vector.tensor_copy(dst, src)
                else:
                    nc.scalar.copy(dst, src)
                k_state["k"] += 1

            out_sb = {}
            for b in range(BL):
                for mi in range(2):
                    out_sb[(b, mi)] = outp.tile(
                        [128, T], MM_DTYPE, tag=f"o{mi}", name=f"o{b}_{mi}"
                    )

            for b in range(BL):
                gvt = gvtp.tile([W78, T], MM_DTYPE, tag="gvt", name=f"gvt{b}")
                gv = {}
                for w in range(4):
                    gv[w] = psp.tile(
                        [128, 2 * T_PS], FP32, tag="pb", name=f"gv{b}_{w}"
                    )

                def quarter(w, q):
                    return gv[w][:W78, q * T_PS : (q + 1) * T_PS]

                # c2 (start=True): packed rows 0:50 (T-half 0) / 64:114 (half 1)
                for w in range(4):
                    p0 = 0 if w < 2 else 64
                    lhs = x2st[b][p0 : p0 + 50, SE0 + 2 * W78 : SE0 + 3 * W78]
                    for q in range(2):
                        nc.tensor.matmul(
                            quarter(w, q),
                            lhs,
                            x2st[b][p0 : p0 + 50, (w % 2) * 2 * T_PS + q * T_PS :
                                    (w % 2) * 2 * T_PS + (q + 1) * T_PS],
                            start=True,
                            stop=False,
                            skip_group_check=True,
                        )
                # zero-weight accumulates: keep the PE activity monitor fed
                # while the c0/c1 reads stream in (adds 0 to gv)
                if b == 0:
                    for s in range(N_WARM):
                        w, q = (s // 2) % 4, s % 2
                        nc.tensor.matmul(
                            quarter(w, q),
                            dummy[:, :W78],
                            dummy[:],
                            start=False,
                            stop=False,
                            skip_group_check=True,
                        )
                # c0 accumulate, per T-half as the split reads land
                for hw in range(2):
                    lhs = x2st[b][0:128, SE0 : SE0 + W78]
                    for w in (2 * hw, 2 * hw + 1):
                        for q in range(2):
                            f0 = w * 2 * T_PS + q * T_PS
                            nc.tensor.matmul(
                                quarter(w, q),
                                lhs,
                                xh[(b, 0)][:, f0 : f0 + T_PS],
                                start=False,
                                stop=False,
                                skip_group_check=True,
                            )
                # c1 (stop) per T-half; evacuate as each half completes
                for hw in range(2):
                    lhs = x2st[b][0:128, SE0 + W78 : SE0 + 2 * W78]
                    for w in (2 * hw, 2 * hw + 1):
                        for q in range(2):
                            f0 = w * 2 * T_PS + q * T_PS
                            nc.tensor.matmul(
                                quarter(w, q),
                                lhs,
                                xh[(b, 1)][:, f0 : f0 + T_PS],
                                start=False,
                                stop=True,
                                skip_group_check=True,
                            )
                    for w in (2 * hw, 2 * hw + 1):
                        evac(
                            gvt[:W78, w * 2 * T_PS : (w + 1) * 2 * T_PS],
                            gv[w][:W78],
                        )

                if b == 0:
                    # tail rows 64:78 of gvt are final output rows 256:270
                    nc.sync.dma_start(out=out_ext[b, 256:M, :], in_=gvt[G:W78, :])

                # mm2 per T-half; gw halves: mi0 lives in x2st[0], mi1 in x2st[1]
                for tt in range(2):
                    for mi in range(2):
                        for h in range(2):
                            o_ps = psp.tile([128, 2 * T_PS], FP32, tag="pb", name="o_ps")
                            c0 = tt * 4 * T_PS + h * 2 * T_PS
                            for q in range(2):
                                nc.tensor.matmul(
                                    o_ps[:, q * T_PS : (q + 1) * T_PS],
                                    x2st[mi][0:G, GE0 : GE0 + 128],
                                    gvt[:G, c0 + q * T_PS : c0 + (q + 1) * T_PS],
                                    start=True,
                                    stop=True,
                                    skip_group_check=True,
                                )
                            evac(out_sb[(b, mi)][:, c0 : c0 + 2 * T_PS], o_ps[:])
                    for mi in range(2):
                        nc.sync.dma_start(
                            out=out_ext[b, mi * 128 : (mi + 1) * 128, tt * XC : (tt + 1) * XC],
                            in_=out_sb[(b, mi)][:, tt * XC : (tt + 1) * XC],
                        )
                if b == 1:
                    nc.sync.dma_start(out=out_ext[b, 256:M, :], in_=gvt[G:W78, :])
    nc.compile()
    return nc


def _host_st(positions, grid_weights):
    """st78 [B, C, 78] f32: bilinear hat weights + folded tail rows."""
    gp = (positions.astype(np.float32) + 1.0) * (GS / 2.0)  # [B, C, 2]
    i = np.arange(GS, dtype=np.float32)
    wy = np.maximum(0.0, 1.0 - np.abs(i[None, None, :] - gp[:, :, 0:1]))
    wx = np.maximum(0.0, 1.0 - np.abs(i[None, None, :] - gp[:, :, 1:2]))
    s = (wy[:, :, :, None] * wx[:, :, None, :]).reshape(B, C, G)
    wtail = s @ grid_weights[256:M].T.astype(np.float32)  # [B, C, 14]
    return np.concatenate([s, wtail], axis=2)


def make_in_maps(x, positions, grid_weights):
    st78 = _host_st(positions, grid_weights)
    gw = np.ascontiguousarray(grid_weights[:256].T).astype(NP_MM)  # [64, 256]
    x_mm = x.astype(NP_MM)
    in_maps = []
    for i in range(N_CORES):
        sl = slice(i * BL, (i + 1) * BL)
        x2st_pack = np.zeros((BL, 128, WX2), dtype=np.float32)
        for b2 in range(BL):
            gb = i * BL + b2
            xc2 = x_mm[gb, 256:C].astype(np.float32).reshape(50, 2, XC)
            x2st_pack[b2, 0:50, 0:XC] = xc2[:, 0]
            x2st_pack[b2, 64:114, 0:XC] = xc2[:, 1]
            x2st_pack[b2, :, SE0 : SE0 + W78] = st78[gb, 0:128]
            x2st_pack[b2, :, SE0 + W78 : SE0 + 2 * W78] = st78[gb, 128:256]
            # c2 block: half-0 weights at rows 0:50, half-1 at rows 64:114
            x2st_pack[b2, 0:50, SE0 + 2 * W78 : SE0 + 3 * W78] = st78[gb, 256:C]
            x2st_pack[b2, 64:114, SE0 + 2 * W78 : SE0 + 3 * W78] = st78[gb, 256:C]
            x2st_pack[b2, 0:64, GE0 : GE0 + 128] = gw[:, b2 * 128 : (b2 + 1) * 128]
        in_maps.append(
            {
                "x2st": x2st_pack.astype(NP_MM),
                "x01": np.ascontiguousarray(x_mm[sl, 0:256]).reshape(BL, 2, 128, T),
            }
        )
    return in_maps


_NC_CACHE = None


def kernel(x, positions, grid_weights):
    global _NC_CACHE
    if _NC_CACHE is None:
        _NC_CACHE = build_nc()
    nc = _NC_CACHE
    in_maps = make_in_maps(x, positions, grid_weights)
    res = run_bass_kernel_spmd(nc, in_maps, core_ids=list(range(N_CORES)))
    out = np.concatenate([r["out"] for r in res.results], axis=0)
    return np.asarray(out, dtype=np.float32)


if __name__ == "__main__":
    xs = np.random.randn(B, C, T).astype(np.float32)
    ps = np.random.uniform(-1, 0.74, (B, C, 2)).astype(np.float32)
    gw = np.random.randn(M, G).astype(np.float32)
    out = kernel(xs, ps, gw)
    print(out.shape, out.dtype)


# revision 11
# speedup vs baseline: 1.0818x; 1.0002x over previous
"""AdaptiveGridMerger Trainium2 kernel.

Math: the reference scatters x[b,c,:] into a flat 8x8 grid with bilinear
(4-corner) weights from positions[b,c,:], then matmuls grid_weights
GW [270,64]. The scatter matrix S_b [64,306] (column c = the bilinear
hat weights of channel c) is tiny and depends only on positions, so it
is built on the HOST. The tail output rows 256:270 are folded into it:
  st78[c, 0:64]  = S_b[:, c]
  st78[c, 64:78] = (S_b.T @ GW[256:270].T)[c]   (Wtail fold)
so mm1 (lhsT=st78) produces gv[0:64] = S@x AND gv[64:78] = out[256:270]
in one pass. mm2 (lhsT=GW[0:256].T) produces out[0:256] from gv[0:64].

Device work: 10 contiguous [128,*] read DMAs on the sync HWDGE ring
(strict FIFO = reads drain at full HBM rate before any write), bf16
matmuls, PSUM->SBUF cast copies alternating DVE/ACT, write DMAs on the
sync ring in readiness order. st and gw ride as extra columns of the
small x2 read (a separate small read is latency-bound and stalls
everything), so the first matmul dependency is the smallest read.

Per batch the contraction accumulates c2 (start, smallest read, first)
-> c0 -> c1 (stop); the big reads are split into [128,2048] halves so
completion semaphores (which lag data by 1.5-3us) hand off earlier.
Zero-weight accumulate matmuls between phases keep the PE activity
monitor busy while reads stream, so the clock stays at 2.4 GHz.
Sharding: data-parallel over batch, 2 batches per core. PSUM: one pool
of 4 x [128,1024] f32 slots = exactly 8 banks.
"""

import numpy as np

import concourse.bass as bass
import concourse.bacc as bacc
import concourse.mybir as mybir
from concourse import tile
from concourse.bass_utils import run_bass_kernel_spmd

B, C, T = 16, 306, 4096
M, G, GS = 270, 64, 8
N_CORES = 8
BL = B // N_CORES  # batches per core

W78 = G + 14          # st block width: 64 grid cols + 14 folded tail cols
XC = T // 2
SE0 = XC              # st base col inside x2st
GE0 = XC + 3 * W78    # gw base col inside x2st
WX2 = GE0 + 128       # x2st width (2048 + 234 + 128 = 2410)
T_PS = 512
N_SPIN = 7
N_WARM = 8

MM_DTYPE = mybir.dt.bfloat16
NP_MM = mybir.dt.np(MM_DTYPE)
FP32 = mybir.dt.float32


def build_nc():
    nc = bacc.Bacc()
    x2st_ext = nc.declare_dram_parameter("x2st", [BL, 128, WX2], MM_DTYPE, isOutput=False)
    x01_ext = nc.declare_dram_parameter("x01", [BL, 2, 128, T], MM_DTYPE, isOutput=False)
    out_ext = nc.declare_dram_parameter("out", [BL, M, T], MM_DTYPE, isOutput=True)

    with tile.TileContext(nc) as tc:
        with (
            tc.tile_pool(name="const", bufs=1) as constp,
            tc.tile_pool(name="xp", bufs=1) as xp,
            tc.tile_pool(name="gvt", bufs=2) as gvtp,
            tc.tile_pool(name="op", bufs=2) as outp,
            tc.tile_pool(name="ps", bufs=4, space=bass.MemorySpace.PSUM) as psp,
        ):
            # PE clock pre-ramp while the first reads stream in.
            dummy = constp.tile([128, T_PS], MM_DTYPE, tag="dummy")
            nc.vector.memset(dummy[:], 0.0)
            spin_ps = psp.tile([128, 2 * T_PS], FP32, tag="pb", name="spin_ps")
            for _ in range(N_SPIN):
                nc.tensor.matmul(
                    spin_ps[:, :T_PS], dummy[:, :128], dummy[:], start=True, stop=True
                )

            # ---- reads on the sync ring, in consumption order; big chunks
            # ---- split into [128,2048] halves for earlier sem handoff
            x2st = {}
            xh = {}
            for b in range(BL):
                x2st[b] = xp.tile([128, WX2], MM_DTYPE, tag=f"x2st{b}", name=f"x2st{b}")
                nc.sync.dma_start(out=x2st[b][:], in_=x2st_ext[b])
                for ci in range(2):
                    t_ = xp.tile([128, T], MM_DTYPE, tag=f"x{b}{ci}", name=f"x{b}{ci}")
                    nc.sync.dma_start(out=t_[:], in_=x01_ext[b, ci])
                    xh[(b, ci)] = t_

            k_state = {"k": 0}

            def evac(dst, src):
                if k_state["k"] % 2 == 0:
                    nc.vector.tensor_copy(dst, src)
                else:
                    nc.scalar.copy(dst, src)
                k_state["k"] += 1

            out_sb = {}
            for b in range(BL):
                for mi in range(2):
                    out_sb[(b, mi)] = outp.tile(
                        [128, T], MM_DTYPE, tag=f"o{mi}", name=f"o{b}_{mi}"
                    )

            for b in range(BL):
                gvt = gvtp.tile([W78, T], MM_DTYPE, tag="gvt", name=f"gvt{b}")
                gv = {}
                for w in range(4):
                    gv[w] = psp.tile(
                        [128, 2 * T_PS], FP32, tag="pb", name=f"gv{b}_{w}"
                    )

                def quarter(w, q):
                    return gv[w][:W78, q * T_PS : (q + 1) * T_PS]

                # c2 (start=True): packed rows 0:50 (T-half 0) / 64:114 (half 1)
                for w in range(4):
                    p0 = 0 if w < 2 else 64
                    lhs = x2st[b][p0 : p0 + 50, SE0 + 2 * W78 : SE0 + 3 * W78]
                    for q in range(2):
                        nc.tensor.matmul(
                            quarter(w, q),
                            lhs,
                            x2st[b][p0 : p0 + 50, (w % 2) * 2 * T_PS + q * T_PS :
                                    (w % 2) * 2 * T_PS + (q + 1) * T_PS],
                            start=True,
                            stop=False,
                            skip_group_check=True,
                        )
                # zero-weight accumulates: keep the PE activity monitor fed
                # while the c0/c1 reads stream in (adds 0 to gv)
                n_warm = N_WARM if b == 0 else 4
                for s in range(n_warm):
                    w, q = (s // 2) % 4, s % 2
                    nc.tensor.matmul(
                        quarter(w, q),
                        dummy[:, :W78],
                        dummy[:],
                        start=False,
                        stop=False,
                        skip_group_check=True,
                    )
                # c0 accumulate, per T-half as the split reads land
                for hw in range(2):
                    lhs = x2st[b][0:128, SE0 : SE0 + W78]
                    for w in (2 * hw, 2 * hw + 1):
                        for q in range(2):
                            f0 = w * 2 * T_PS + q * T_PS
                            nc.tensor.matmul(
                                quarter(w, q),
                                lhs,
                                xh[(b, 0)][:, f0 : f0 + T_PS],
                                start=False,
                                stop=False,
                                skip_group_check=True,
                            )
                # c1 (stop) per T-half; evacuate as each half completes
                for hw in range(2):
                    lhs = x2st[b][0:128, SE0 + W78 : SE0 + 2 * W78]
                    for w in (2 * hw, 2 * hw + 1):
                        for q in range(2):
                            f0 = w * 2 * T_PS + q * T_PS
                            nc.tensor.matmul(
                                quarter(w, q),
                                lhs,
                                xh[(b, 1)][:, f0 : f0 + T_PS],
                                start=False,
                                stop=True,
                                skip_group_check=True,
                            )
                    for w in (2 * hw, 2 * hw + 1):
                        evac(
                            gvt[:W78, w * 2 * T_PS : (w + 1) * 2 * T_PS],
                            gv[w][:W78],
                        )

                if b == 0:
                    # tail rows 64:78 of gvt are final output rows 256:270
                    nc.sync.dma_start(out=out_ext[b, 256:M, :], in_=gvt[G:W78, :])

                # mm2 per T-half; gw halves: mi0 lives in x2st[0], mi1 in x2st[1]
                for tt in range(2):
                    for mi in range(2):
                        for h in range(2):
                            o_ps = psp.tile([128, 2 * T_PS], FP32, tag="pb", name="o_ps")
                            c0 = tt * 4 * T_PS + h * 2 * T_PS
                            for q in range(2):
                                nc.tensor.matmul(
                                    o_ps[:, q * T_PS : (q + 1) * T_PS],
                                    x2st[mi][0:G, GE0 : GE0 + 128],
                                    gvt[:G, c0 + q * T_PS : c0 + (q + 1) * T_PS],
                                    start=True,
                                    stop=True,
                                    skip_group_check=True,
                                )
                            evac(out_sb[(b, mi)][:, c0 : c0 + 2 * T_PS], o_ps[:])
                    for mi in range(2):
                        nc.sync.dma_start(
                            out=out_ext[b, mi * 128 : (mi + 1) * 128, tt * XC : (tt + 1) * XC],
                            in_=out_sb[(b, mi)][:, tt * XC : (tt + 1) * XC],
                        )
                if b == 1:
                    nc.sync.dma_start(out=out_ext[b, 256:M, :], in_=gvt[G:W78, :])
    nc.compile()
    return nc


def _host_st(positions, grid_weights):
    """st78 [B, C, 78] f32: bilinear hat weights + folded tail rows."""
    gp = (positions.astype(np.float32) + 1.0) * (GS / 2.0)  # [B, C, 2]
    i = np.arange(GS, dtype=np.float32)
    wy = np.maximum(0.0, 1.0 - np.abs(i[None, None, :] - gp[:, :, 0:1]))
    wx = np.maximum(0.0, 1.0 - np.abs(i[None, None, :] - gp[:, :, 1:2]))
    s = (wy[:, :, :, None] * wx[:, :, None, :]).reshape(B, C, G)
    wtail = s @ grid_weights[256:M].T.astype(np.float32)  # [B, C, 14]
    return np.concatenate([s, wtail], axis=2)


def make_in_maps(x, positions, grid_weights):
    st78 = _host_st(positions, grid_weights)
    gw = np.ascontiguousarray(grid_weights[:256].T).astype(NP_MM)  # [64, 256]
    x_mm = x.astype(NP_MM)
    in_maps = []
    for i in range(N_CORES):
        sl = slice(i * BL, (i + 1) * BL)
        x2st_pack = np.zeros((BL, 128, WX2), dtype=np.float32)
        for b2 in range(BL):
            gb = i * BL + b2
            xc2 = x_mm[gb, 256:C].astype(np.float32).reshape(50, 2, XC)
            x2st_pack[b2, 0:50, 0:XC] = xc2[:, 0]
            x2st_pack[b2, 64:114, 0:XC] = xc2[:, 1]
            x2st_pack[b2, :, SE0 : SE0 + W78] = st78[gb, 0:128]
            x2st_pack[b2, :, SE0 + W78 : SE0 + 2 * W78] = st78[gb, 128:256]
            # c2 block: half-0 weights at rows 0:50, half-1 at rows 64:114
            x2st_pack[b2, 0:50, SE0 + 2 * W78 : SE0 + 3 * W78] = st78[gb, 256:C]
            x2st_pack[b2, 64:114, SE0 + 2 * W78 : SE0 + 3 * W78] = st78[gb, 256:C]
            x2st_pack[b2, 0:64, GE0 : GE0 + 128] = gw[:, b2 * 128 : (b2 + 1) * 128]
        in_maps.append(
            {
                "x2st": x2st_pack.astype(NP_MM),
                "x01": np.ascontiguousarray(x_mm[sl, 0:256]).reshape(BL, 2, 128, T),
            }
        )
    return in_maps


_NC_CACHE = None


def kernel(x, positions, grid_weights):
    global _NC_CACHE
    if _NC_CACHE is None:
        _NC_CACHE = build_nc()
    nc = _NC_CACHE
    in_maps = make_in_maps(x, positions, grid_weights)
    res = run_bass_kernel_spmd(nc, in_maps, core_ids=list(range(N_CORES)))
    out = np.concatenate([r["out"] for r in res.results], axis=0)
    return np.asarray(out, dtype=np.float32)


if __name__ == "__main__":
    xs = np.random.randn(B, C, T).astype(np.float32)
    ps = np.random.uniform(-1, 0.74, (B, C, 2)).astype(np.float32)
    gw = np.random.randn(M, G).astype(np.float32)
    out = kernel(xs, ps, gw)
    print(out.shape, out.dtype)


# revision 12
# speedup vs baseline: 1.1520x; 1.0649x over previous
"""AdaptiveGridMerger Trainium2 kernel.

Math: the reference scatters x[b,c,:] into a flat 8x8 grid with bilinear
(4-corner) weights from positions[b,c,:], then matmuls grid_weights
GW [270,64]. The scatter matrix S_b [64,306] (column c = the bilinear
hat weights of channel c) is tiny and depends only on positions, so it
is built on the HOST. The tail output rows 256:270 are folded into it:
  st78[c, 0:64]  = S_b[:, c]
  st78[c, 64:78] = (S_b.T @ GW[256:270].T)[c]   (Wtail fold)
so mm1 (lhsT=st78) produces gv[0:64] = S@x AND gv[64:78] = out[256:270]
in one pass. mm2 (lhsT=GW[0:256].T) produces out[0:256] from gv[0:64].

Device work: 6 contiguous [128,*] read DMAs on the sync HWDGE ring
(strict FIFO = reads drain at full HBM rate before any write), bf16
matmuls, PSUM->SBUF cast copies alternating DVE/ACT, write DMAs on the
sync ring in readiness order. st and gw ride as extra columns of the
small c2 read, so the first matmul dependency is the smallest read.

The kernel is paced by the 2-wide PSUM-evacuation stream (24 x ~1.2us
copies), so the schedule keeps it dense: batch 0 accumulates c2
(start) -> c0 -> c1 (stop) and evacuates per T-half; batch 1's mm1 is
split into two T-half phases (2 PSUM slots each) braided between the
copy-paced mm2 halves of batch 0 so the copy queue never starves.
Zero-weight accumulate matmuls plug PE idle windows so the activity
monitor keeps the clock at 2.4 GHz. Sharding: data-parallel over
batch, 2 batches per core. PSUM: 4 x [128,1024] f32 slots = 8 banks.
"""

import numpy as np

import concourse.bass as bass
import concourse.bacc as bacc
import concourse.mybir as mybir
from concourse import tile
from concourse.bass_utils import run_bass_kernel_spmd

B, C, T = 16, 306, 4096
M, G, GS = 270, 64, 8
N_CORES = 8
BL = B // N_CORES  # batches per core

W78 = G + 14          # st block width: 64 grid cols + 14 folded tail cols
XC = T // 2
SE0 = XC              # st base col inside x2st
GE0 = XC + 3 * W78    # gw base col inside x2st
WX2 = GE0 + 128       # x2st width (2048 + 234 + 128 = 2410)
T_PS = 512
N_SPIN = 7

MM_DTYPE = mybir.dt.bfloat16
NP_MM = mybir.dt.np(MM_DTYPE)
FP32 = mybir.dt.float32


def build_nc():
    nc = bacc.Bacc()
    x2st_ext = nc.declare_dram_parameter("x2st", [BL, 128, WX2], MM_DTYPE, isOutput=False)
    x01_ext = nc.declare_dram_parameter("x01", [BL, 2, 128, T], MM_DTYPE, isOutput=False)
    out_ext = nc.declare_dram_parameter("out", [BL, M, T], MM_DTYPE, isOutput=True)

    with tile.TileContext(nc) as tc:
        with (
            tc.tile_pool(name="const", bufs=1) as constp,
            tc.tile_pool(name="xp", bufs=1) as xp,
            tc.tile_pool(name="gvt", bufs=2) as gvtp,
            tc.tile_pool(name="op", bufs=2) as outp,
            tc.tile_pool(name="ps", bufs=4, space=bass.MemorySpace.PSUM) as psp,
        ):
            # PE clock pre-ramp while the first reads stream in.
            dummy = constp.tile([128, T_PS], MM_DTYPE, tag="dummy")
            nc.vector.memset(dummy[:], 0.0)
            spin_ps = psp.tile([128, 2 * T_PS], FP32, tag="pb", name="spin_ps")
            for _ in range(N_SPIN):
                nc.tensor.matmul(
                    spin_ps[:, :T_PS], dummy[:, :128], dummy[:], start=True, stop=True
                )

            # ---- reads on the sync ring, in consumption order
            x2st = {}
            xh = {}
            for b in range(BL):
                x2st[b] = xp.tile([128, WX2], MM_DTYPE, tag=f"x2st{b}", name=f"x2st{b}")
                nc.sync.dma_start(out=x2st[b][:], in_=x2st_ext[b])
                for ci in range(2):
                    t_ = xp.tile([128, T], MM_DTYPE, tag=f"x{b}{ci}", name=f"x{b}{ci}")
                    nc.sync.dma_start(out=t_[:], in_=x01_ext[b, ci])
                    xh[(b, ci)] = t_

            k_state = {"k": 0}

            def evac(dst, src):
                if k_state["k"] % 2 == 0:
                    nc.vector.tensor_copy(dst, src)
                else:
                    nc.scalar.copy(dst, src)
                k_state["k"] += 1

            out_sb = {}
            gvts = {}
            for b in range(BL):
                gvts[b] = gvtp.tile([W78, T], MM_DTYPE, tag="gvt", name=f"gvt{b}")
                for mi in range(2):
                    out_sb[(b, mi)] = outp.tile(
                        [128, T], MM_DTYPE, tag=f"o{mi}", name=f"o{b}_{mi}"
                    )

            gv = {}  # (b, w) -> live psum accumulator

            def quarter(b, w, q):
                return gv[(b, w)][:W78, q * T_PS : (q + 1) * T_PS]

            def mm1_chunk(b, waves, which, start, stop):
                """Accumulate chunk `which` into the gv tiles of `waves`."""
                for w in waves:
                    if (b, w) not in gv:
                        gv[(b, w)] = psp.tile(
                            [128, 2 * T_PS], FP32, tag="pb", name=f"gv{b}_{w}"
                        )
                for w in waves:
                    if which == 2:
                        p0 = 0 if w < 2 else 64
                        lhs = x2st[b][p0 : p0 + 50, SE0 + 2 * W78 : SE0 + 3 * W78]
                        for q in range(2):
                            f0 = (w % 2) * 2 * T_PS + q * T_PS
                            nc.tensor.matmul(
                                quarter(b, w, q),
                                lhs,
                                x2st[b][p0 : p0 + 50, f0 : f0 + T_PS],
                                start=start, stop=stop, skip_group_check=True,
                            )
                    else:
                        lhs = x2st[b][0:128, SE0 + which * W78 : SE0 + (which + 1) * W78]
                        for q in range(2):
                            f0 = w * 2 * T_PS + q * T_PS
                            nc.tensor.matmul(
                                quarter(b, w, q),
                                lhs,
                                xh[(b, which)][:, f0 : f0 + T_PS],
                                start=start, stop=stop, skip_group_check=True,
                            )

            def warm(b, waves, n):
                """Zero-weight accumulates keep the PE activity monitor fed."""
                for s in range(n):
                    w = waves[(s // 2) % len(waves)]
                    nc.tensor.matmul(
                        quarter(b, w, s % 2),
                        dummy[:, :W78],
                        dummy[:],
                        start=False, stop=False, skip_group_check=True,
                    )

            def evac_waves(b, waves):
                for w in waves:
                    evac(
                        gvts[b][:W78, w * 2 * T_PS : (w + 1) * 2 * T_PS],
                        gv[(b, w)][:W78],
                    )
                    del gv[(b, w)]

            def mm2_half(b, tt):
                for mi in range(2):
                    for h in range(2):
                        o_ps = psp.tile([128, 2 * T_PS], FP32, tag="pb", name="o_ps")
                        c0 = tt * 4 * T_PS + h * 2 * T_PS
                        for q in range(2):
                            nc.tensor.matmul(
                                o_ps[:, q * T_PS : (q + 1) * T_PS],
                                x2st[mi][0:G, GE0 : GE0 + 128],
                                gvts[b][:G, c0 + q * T_PS : c0 + (q + 1) * T_PS],
                                start=True, stop=True, skip_group_check=True,
                            )
                        evac(out_sb[(b, mi)][:, c0 : c0 + 2 * T_PS], o_ps[:])
                for mi in range(2):
                    nc.sync.dma_start(
                        out=out_ext[b, mi * 128 : (mi + 1) * 128, tt * XC : (tt + 1) * XC],
                        in_=out_sb[(b, mi)][:, tt * XC : (tt + 1) * XC],
                    )

            # ---- batch 0: full mm1, evac per T-half
            mm1_chunk(0, (0, 1, 2, 3), 2, True, False)
            warm(0, (0, 1, 2, 3), 8)
            mm1_chunk(0, (0, 1, 2, 3), 0, False, False)
            mm1_chunk(0, (0, 1), 1, False, True)
            evac_waves(0, (0, 1))
            mm1_chunk(0, (2, 3), 1, False, True)
            evac_waves(0, (2, 3))
            nc.sync.dma_start(out=out_ext[0, 256:M, :], in_=gvts[0][G:W78, :])

            # ---- braid: mm2(b0) halves alternate with b1's mm1 halves
            mm2_half(0, 0)
            mm1_chunk(1, (0, 1), 2, True, False)
            warm(1, (0, 1), 4)
            mm1_chunk(1, (0, 1), 0, False, False)
            mm1_chunk(1, (0, 1), 1, False, True)
            evac_waves(1, (0, 1))
            mm2_half(0, 1)
            mm1_chunk(1, (2, 3), 2, True, False)
            mm1_chunk(1, (2, 3), 0, False, False)
            mm1_chunk(1, (2, 3), 1, False, True)
            evac_waves(1, (2, 3))
            mm2_half(1, 0)
            mm2_half(1, 1)
            nc.sync.dma_start(out=out_ext[1, 256:M, :], in_=gvts[1][G:W78, :])
    nc.compile()
    return nc


def _host_st(positions, grid_weights):
    """st78 [B, C, 78] f32: bilinear hat weights + folded tail rows."""
    gp = (positions.astype(np.float32) + 1.0) * (GS / 2.0)  # [B, C, 2]
    i = np.arange(GS, dtype=np.float32)
    wy = np.maximum(0.0, 1.0 - np.abs(i[None, None, :] - gp[:, :, 0:1]))
    wx = np.maximum(0.0, 1.0 - np.abs(i[None, None, :] - gp[:, :, 1:2]))
    s = (wy[:, :, :, None] * wx[:, :, None, :]).reshape(B, C, G)
    wtail = s @ grid_weights[256:M].T.astype(np.float32)  # [B, C, 14]
    return np.concatenate([s, wtail], axis=2)


def make_in_maps(x, positions, grid_weights):
    st78 = _host_st(positions, grid_weights)
    gw = np.ascontiguousarray(grid_weights[:256].T).astype(NP_MM)  # [64, 256]
    x_mm = x.astype(NP_MM)
    in_maps = []
    for i in range(N_CORES):
        sl = slice(i * BL, (i + 1) * BL)
        x2st_pack = np.zeros((BL, 128, WX2), dtype=np.float32)
        for b2 in range(BL):
            gb = i * BL + b2
            xc2 = x_mm[gb, 256:C].astype(np.float32).reshape(50, 2, XC)
            x2st_pack[b2, 0:50, 0:XC] = xc2[:, 0]
            x2st_pack[b2, 64:114, 0:XC] = xc2[:, 1]
            x2st_pack[b2, :, SE0 : SE0 + W78] = st78[gb, 0:128]
            x2st_pack[b2, :, SE0 + W78 : SE0 + 2 * W78] = st78[gb, 128:256]
            # c2 block: half-0 weights at rows 0:50, half-1 at rows 64:114
            x2st_pack[b2, 0:50, SE0 + 2 * W78 : SE0 + 3 * W78] = st78[gb, 256:C]
            x2st_pack[b2, 64:114, SE0 + 2 * W78 : SE0 + 3 * W78] = st78[gb, 256:C]
            x2st_pack[b2, 0:64, GE0 : GE0 + 128] = gw[:, b2 * 128 : (b2 + 1) * 128]
        in_maps.append(
            {
                "x2st": x2st_pack.astype(NP_MM),
                "x01": np.ascontiguousarray(x_mm[sl, 0:256]).reshape(BL, 2, 128, T),
            }
        )
    return in_maps


_NC_CACHE = None


def kernel(x, positions, grid_weights):
    global _NC_CACHE
    if _NC_CACHE is None:
        _NC_CACHE = build_nc()
    nc = _NC_CACHE
    in_maps = make_in_maps(x, positions, grid_weights)
    res = run_bass_kernel_spmd(nc, in_maps, core_ids=list(range(N_CORES)))
    out = np.concatenate([r["out"] for r in res.results], axis=0)
    return np.asarray(out, dtype=np.float32)


if __name__ == "__main__":
    xs = np.random.randn(B, C, T).astype(np.float32)
    ps = np.random.uniform(-1, 0.74, (B, C, 2)).astype(np.float32)
    gw = np.random.randn(M, G).astype(np.float32)
    out = kernel(xs, ps, gw)
    print(out.shape, out.dtype)
